# revision 20
# baseline (speedup 1.0000x reference)
"""CPA-loss kernel for Trainium2, data-parallel over 8 NeuronCores.

Math (per batch row b with target class c = targets[b]):
    e[j]  = exp(logits[b, j])            (no max-shift; |logits| <~ 6 so exp is safe,
                                          and the shift cancels in sigma up to an
                                          EPS-scaling that is ~1e-7 relative)
    den   = sum_j GF[c, j] * e[j]        (GF diag == 1 makes this equal the reference
                                          ((1-t)e) @ GF.T + e at column c)
    sigma = e[c] / (den + EPS)
    loss  = mean_b( -pf[c] * log(sigma + EPS) ),  pf = (1+TAU)/(cos(lp,gp)+TAU)

Device strategy per core (B/8 = 16384 rows, 8 super-tiles of [128p, 16tau, 128c]):
  the per-row "gather" of logGF rows runs on the PE with one-hot stationaries,
  in bf16 hi/lo pairs (exact one-hots, hi/lo-split tables) so matmuls run at
  1 cycle/column with fast weight loads:
    T^T[i, k]  = (targets[k] == i)            DVE is_equal on broadcast int16
    MM1a (lhsT=T^T, rhs=[logGF_hi | 14*I]):   PSUM[b, clean] = logGF_hi[c_b, :]
                                              PSUM[b, spike] = 14*onehot(c_b)
    MM1b (lhsT=T^T, rhs=logGF_lo):            PSUM[b, clean] += logGF_lo[c_b, :]
    MM2a/b (lhsT=I, rhs=[l_hi|l_hi],[l_lo|l_lo]): PSUM[b, :] += [logits | logits]
  then per tile / quarter-super-tile:
    ACT  exp(PSUM clean) with accum_out  -> den[b]   (fused exp+row-sum)
    DVE  reduce_max X (PSUM spike half)  -> l_sel+14 (exact: spike dominates)
  final phase on [128, 128] column buffers:
    e_sel = exp(max - 14);  sigma = e_sel/(den+EPS);  -pf * ln(sigma+EPS) summed.
pf[targets[b]] is a 128-entry-table lookup -> marshaled on host. Host sums the 8
per-core [128,1] partials (exact mean + sign).
"""

import ml_dtypes
import numpy as np

import concourse.bacc as bacc
import concourse.bass as bass
import concourse.tile as tile
from concourse import mybir
from concourse.bass_utils import run_bass_kernel_spmd

B, C, D = 131072, 128, 64
N_CORES = 8
B_CORE = B // N_CORES  # 16384
ST = 8                 # super-tiles per core
TPS = 16               # tiles (128 rows each) per super-tile
HT = 4                 # tiles per PSUM group (2 banks)
ROWS_ST = 128 * TPS    # 2048
TAU = 3.0
EPS = 1e-6
SPIKE = 14.0           # exp-domain spike: l_sel + 14 always wins the row max

F32 = mybir.dt.float32
BF16 = mybir.dt.bfloat16
I16 = mybir.dt.int16
I8 = mybir.dt.int8
BF = ml_dtypes.bfloat16

_CACHE = {}


def _build_program():
    nc = bacc.Bacc("TRN2", target_bir_lowering=False, debug=False)

    lhl_d = nc.dram_tensor("logits_hl", [B_CORE, 2, C], BF16, kind="ExternalInput")
    targets16_d = nc.dram_tensor("targets16", [B_CORE], I16, kind="ExternalInput")
    gfp_hi_d = nc.dram_tensor("gfp_hi", [C, 2 * C], BF16, kind="ExternalInput")
    gfp_lo_d = nc.dram_tensor("gfp_lo", [C, C], BF16, kind="ExternalInput")
    ident_d = nc.dram_tensor("ident", [128, 128], BF16, kind="ExternalInput")
    # pf[targets[b]] pre-permuted to [p, st*TPS + tau] (b = st*2048 + p*16 + tau)
    pfsel_d = nc.dram_tensor("pfsel", [128, ST * TPS], F32, kind="ExternalInput")
    out_d = nc.dram_tensor("out", [128, 1], F32, kind="ExternalOutput")

    add = mybir.AluOpType.add
    mult = mybir.AluOpType.mult
    is_equal = mybir.AluOpType.is_equal
    AX = mybir.ActivationFunctionType

    with tile.TileContext(nc) as tc:
        with (
            tc.tile_pool(name="singles", bufs=1) as singles,
            tc.tile_pool(name="lp", bufs=4) as lp,
            tc.tile_pool(name="tp", bufs=3) as tp,
            tc.tile_pool(name="ep", bufs=8) as ep,
            tc.tile_pool(name="psum", bufs=4, space="PSUM") as pp,
        ):
            # ---- one-time constants ----
            gfp_hi_sb = singles.tile([128, 2 * C], BF16)
            nc.sync.dma_start(out=gfp_hi_sb[:], in_=gfp_hi_d.ap())
            gfp_lo_sb = singles.tile([128, C], BF16)
            nc.sync.dma_start(out=gfp_lo_sb[:], in_=gfp_lo_d.ap())
            ident_sb = singles.tile([128, 128], BF16)
            nc.sync.dma_start(out=ident_sb[:], in_=ident_d.ap())
            pfsel_sb = singles.tile([128, ST, TPS], F32)
            nc.sync.dma_start(
                out=pfsel_sb[:],
                in_=pfsel_d.ap().rearrange("p (st t) -> p st t", st=ST, t=TPS),
            )
            iota_p = singles.tile([128, 1], I16)
            nc.gpsimd.iota(iota_p[:], pattern=[[1, 1]], base=0, channel_multiplier=1)
            iota_flat = singles.tile([128, ROWS_ST], I16)
            nc.vector.tensor_copy(
                iota_flat[:], iota_p[:].to_broadcast([128, ROWS_ST])
            )

            den_all = singles.tile([128, ST, TPS], F32)
            max_all = singles.tile([128, ST, TPS], F32)

            # tile tau covers rows b = st*2048 + p*16 + tau (p = out partition),
            # so each partition's logits DMA span is contiguous (16 rows)
            lhl_t = lhl_d.ap().rearrange(
                "(st p g) two c -> st p g two c", st=ST, p=128, g=TPS
            )

            for st in range(ST):
                lhl = lp.tile([128, TPS, 2, C], BF16)
                nc.sync.dma_start(out=lhl[:], in_=lhl_t[st])

                # targets of this super-tile broadcast to all 128 partitions
                trep = tp.tile([128, ROWS_ST], I16)
                nc.sync.dma_start(
                    out=trep[:],
                    in_=bass.AP(
                        tensor=targets16_d,
                        offset=st * ROWS_ST,
                        ap=[[0, 128], [1, ROWS_ST]],
                    ),
                )
                # T^T[i, k] = (targets[st*2048+k] == i), k = p*16 + tau
                tt = tp.tile([128, ROWS_ST], BF16)
                nc.vector.tensor_tensor(tt[:], trep[:], iota_flat[:], op=is_equal)
                ttv = tt[:].rearrange("i (p g) -> i p g", g=TPS)

                for h in range(TPS // HT):
                    # [b-part, tile, {clean|spike}, c] — tiles 2k,2k+1 share a
                    # PSUM bank: only the bank's first MM starts the zero
                    # region, only its last MM stops it.
                    gp = pp.tile([128, HT, 2, C], F32)
                    for tt_i in range(HT):
                        tau = h * HT + tt_i
                        nc.tensor.matmul(
                            gp[:, tt_i, :, :],
                            lhsT=ttv[:, :, tau],
                            rhs=gfp_hi_sb[:],
                            start=(tt_i % 2 == 0),
                            stop=False,
                        )
                        nc.tensor.matmul(
                            gp[:, tt_i, 0, :],
                            lhsT=ttv[:, :, tau],
                            rhs=gfp_lo_sb[:],
                            start=False,
                            stop=False,
                        )
                    # one identity-MM pair per PSUM bank injects [l | l]
                    for bk in range(HT // 2):
                        tau0 = h * HT + 2 * bk
                        for li in (0, 1):
                            base = lhl[:, tau0, li, :]
                            nc.tensor.matmul(
                                gp[:, 2 * bk : 2 * bk + 2, :, :],
                                lhsT=ident_sb[:],
                                rhs=bass.AP(
                                    tensor=base.tensor,
                                    offset=base.offset,
                                    ap=[base.ap[0], [2 * C, 2], [0, 2], [1, C]],
                                ),
                                start=False,
                                stop=(li == 1),
                            )
                    # den path: e = exp(l + logGF) batched, then row-sums on DVE
                    et = ep.tile([128, HT, C], F32)
                    nc.scalar.activation(et[:], gp[:, :, 0, :], AX.Exp)
                    nc.vector.tensor_reduce(
                        den_all[:, st, h * HT : (h + 1) * HT],
                        et[:],
                        axis=mybir.AxisListType.X,
                        op=add,
                    )
                    # select path: row max of (l + SPIKE*onehot) = l_sel + SPIKE
                    nc.vector.tensor_reduce(
                        max_all[:, st, h * HT : (h + 1) * HT],
                        gp[:, :, 1, :],
                        axis=mybir.AxisListType.X,
                        op=mybir.AluOpType.max,
                    )

            # ---- final phase on [128, 128] ----
            neg_spike = singles.tile([128, 1], F32)
            nc.vector.memset(neg_spike[:], -SPIKE)
            eps_bias = singles.tile([128, 1], F32)
            nc.vector.memset(eps_bias[:], EPS)

            e_sel = singles.tile([128, ST, TPS], F32)
            nc.scalar.activation(e_sel[:], max_all[:], AX.Exp, bias=neg_spike[:])
            nc.vector.tensor_scalar_add(den_all[:], den_all[:], EPS)
            rec = singles.tile([128, ST, TPS], F32)
            nc.vector.reciprocal(rec[:], den_all[:])
            nc.vector.tensor_tensor(e_sel[:], e_sel[:], rec[:], op=mult)
            nc.scalar.activation(e_sel[:], e_sel[:], AX.Ln, bias=eps_bias[:])
            wv = singles.tile([128, ST, TPS], F32)
            row_part = singles.tile([128, 1], F32)
            nc.vector.scalar_tensor_tensor(
                out=wv[:],
                in0=e_sel[:],
                scalar=1.0,
                in1=pfsel_sb[:],
                op0=mult,
                op1=mult,
                accum_out=row_part[:],
            )
            nc.sync.dma_start(out=out_d.ap(), in_=row_part[:])

    nc.compile()
    return nc


def _host_tables(local_proto, global_proto, global_factor):
    lp = np.asarray(local_proto, dtype=np.float64)
    gp = np.asarray(global_proto, dtype=np.float64)
    gf = np.asarray(global_factor, dtype=np.float64)
    cos = (lp * gp).sum(-1) / (
        np.linalg.norm(lp, axis=-1) * np.linalg.norm(gp, axis=-1) + EPS
    )
    pf = ((1.0 + TAU) / (cos + TAU)).astype(np.float32)
    lgf = np.log(gf).astype(np.float32)
    lgf_hi = lgf.astype(BF)
    lgf_lo = (lgf - lgf_hi.astype(np.float32)).astype(BF)
    gfp_hi = np.zeros((C, 2 * C), dtype=BF)
    gfp_hi[:, :C] = lgf_hi
    gfp_hi[:, C:] = (SPIKE * np.eye(C, dtype=np.float32)).astype(BF)
    return gfp_hi, np.ascontiguousarray(lgf_lo), pf


def _run(logits, targets, local_proto, global_proto, global_factor, trace=False):
    if "nc" not in _CACHE:
        _CACHE["nc"] = _build_program()
    nc = _CACHE["nc"]

    logits = np.ascontiguousarray(np.asarray(logits, dtype=np.float32))
    targets = np.asarray(targets, dtype=np.int32)
    gfp_hi, gfp_lo, pf = _host_tables(local_proto, global_proto, global_factor)
    targets16 = np.ascontiguousarray(targets.astype(np.int16))
    ident = np.eye(128, dtype=np.float32).astype(BF)
    l_hl = np.empty((B, 2, C), dtype=BF)
    l_hl[:, 0, :] = logits.astype(BF)
    l_hl[:, 1, :] = (logits - l_hl[:, 0, :].astype(np.float32)).astype(BF)

    in_maps = []
    for k in range(N_CORES):
        sl = slice(k * B_CORE, (k + 1) * B_CORE)
        # pf[targets] permuted to [p, st*TPS+tau]: b = st*2048 + p*16 + tau
        pfs = pf[targets[sl]].reshape(ST, 128, TPS).transpose(1, 0, 2)
        in_maps.append(
            {
                "logits_hl": np.ascontiguousarray(l_hl[sl]),
                "targets16": targets16[sl],
                "gfp_hi": gfp_hi,
                "gfp_lo": gfp_lo,
                "ident": ident,
                "pfsel": np.ascontiguousarray(pfs.reshape(128, ST * TPS)),
            }
        )
    res = run_bass_kernel_spmd(
        nc, in_maps, core_ids=list(range(N_CORES)), trace=trace
    )
    total = 0.0
    for r in res.results:
        total += float(np.asarray(r["out"], dtype=np.float64).sum())
    loss = np.float32(-total / B)
    return np.asarray(loss, dtype=np.float32), res


def kernel(logits, targets, local_proto, global_proto, global_factor):
    out, _ = _run(logits, targets, local_proto, global_proto, global_factor)
    return out
